# revision 19
# baseline (speedup 1.0000x reference)
"""Embedding lookup (gather of rows) distributed over 8 NeuronCores.

Full problem: x:[1, 8192] int token ids, weights:[50257, 768] f32.
Output: weights[x[0]] -> [8192, 768] f32.

Sharding: data-parallel over the sequence dim. Each of the 8 cores gets
1024 token ids plus a full replica of the embedding table and gathers its
own rows with indirect DMA (DRAM -> SBUF; the HW consumes one index per
partition per indirect DMA, so each gather moves 128 rows), then writes
its contiguous [1024, 768] output slice back to DRAM. No collectives;
the host concatenates the 8 slices.

Raw Bass, no Block(): the engine streams (SP: HWDGE idx load + half the
write-outs; ACT: the other write-outs; gpsimd: the SWDGE gathers, split
over two SWDGE queues) are emitted straight into the main block with
manual semaphores and no end-of-kernel all-engine barrier. Semaphore
reset for re-execution is provided by the runtime's own postamble (it
zeroes all 256 semaphores after every execution).
"""

import numpy as np

import concourse.bass as bass
import concourse.mybir as mybir
from concourse import bacc
from concourse.bass_utils import run_bass_kernel_spmd

VOCAB = 50257
EMBED = 768
SEQ = 8192
N_CORES = 8
TOK_PER_CORE = SEQ // N_CORES  # 1024
J = TOK_PER_CORE // 128  # 8 gathers of 128 rows each


def build_nc():
    nc = bacc.Bacc(
        "TRN2",
        target_bir_lowering=False,
        debug=False,
        num_devices=N_CORES,
        num_swdge_queues=2,
    )
    idx = nc.dram_tensor(
        "idx", [1, TOK_PER_CORE], mybir.dt.int32, kind="ExternalInput"
    )
    w = nc.dram_tensor("w", [VOCAB, EMBED], mybir.dt.float32, kind="ExternalInput")
    out = nc.dram_tensor(
        "out", [TOK_PER_CORE, EMBED], mybir.dt.float32, kind="ExternalOutput"
    )
    # idx arrives pre-transposed from the host: idx[8p+j] = x_shard[j*128+p],
    # so idx_sb[p, j] (contiguous load) indexes token j*128+p.
    idx_re = idx.ap().rearrange("a (p j) -> (a p) j", p=128)

    with (
        nc.sbuf_tensor("idx_sb", [128, J], mybir.dt.int32) as idx_sb,
        nc.sbuf_tensor("gbuf", [128, J * EMBED], mybir.dt.float32) as gbuf,
        nc.semaphore("isem") as isem,
        nc.semaphore("wsem") as wsem,
    ):
        gsems = [nc.alloc_semaphore(f"gsem{g}") for g in range(J)]

        gp = nc.gpsimd
        sp = nc.sync

        # idx load on SP (HWDGE: lower first-byte latency); gathers on gpsimd.
        sp.dma_start(idx_sb[:], idx_re).then_inc(isem, 16)
        gp.wait_ge(isem, 16)
        for j in range(J):
            bi = gp.indirect_dma_start(
                out=gbuf[:, j * EMBED : (j + 1) * EMBED],
                out_offset=None,
                in_=w.ap(),
                in_offset=bass.IndirectOffsetOnAxis(ap=idx_sb[:, j : j + 1], axis=0),
            )
            bi.then_inc(gsems[j], 16)
            if j % 2 == 1:
                # Alternate gathers onto the second SWDGE queue so the two
                # descriptor rings feed the SDMA engines in parallel.
                (bi.instruction if hasattr(bi, "instruction") else bi).queue = (
                    "qPoolDynamic1"
                )

        # Write-outs alternate between the two HWDGE queues (SP and ACT) so
        # descriptor rings and completion receipts don't pile on one queue.
        # With the host-side id transpose (see kernel()), gather j holds
        # tokens j*128..j*128+127, so each write is one fully contiguous
        # 393KB block of the output.
        act = nc.scalar
        for j in range(J):
            eng = sp if j % 2 == 0 else act
            eng.wait_ge(gsems[j], 16)
            eng.dma_start(
                out.ap()[j * 128 : (j + 1) * 128, :],
                gbuf[:, j * EMBED : (j + 1) * EMBED],
            ).then_inc(wsem, 16)
        sp.wait_ge(wsem, J * 16)

    # Drop the const-AP prime memsets Bass emits unconditionally in its
    # prologue — nothing in this kernel reads them, and they extend the
    # measured kernel span at the front.
    entry = nc.m.functions[0].blocks[0]
    dead = [
        i
        for i in entry.instructions
        if isinstance(i, mybir.InstMemset)
        and i.outs
        and str(getattr(i.outs[0], "memref", "")).startswith("const-")
    ]
    for i in dead:
        entry.instructions.remove(i)

    nc.compile()
    return nc


def kernel(x, weights):
    x_np = np.ascontiguousarray(np.asarray(x).reshape(-1).astype(np.int32))
    w_np = np.ascontiguousarray(np.asarray(weights), dtype=np.float32)
    assert x_np.shape == (SEQ,) and w_np.shape == (VOCAB, EMBED)

    nc = build_nc()
    # Fixed per-core layout permutation (value-independent): hand the core
    # its ids as the 8x128 transpose so device writes are block-contiguous.
    in_maps = [
        {
            "idx": np.ascontiguousarray(
                x_np[k * TOK_PER_CORE : (k + 1) * TOK_PER_CORE]
                .reshape(J, 128)
                .T
            ).reshape(1, TOK_PER_CORE),
            "w": w_np,
        }
        for k in range(N_CORES)
    ]
    res = run_bass_kernel_spmd(nc, in_maps, core_ids=list(range(N_CORES)))
    return np.concatenate([r["out"] for r in res.results], axis=0)


# revision 20
# speedup vs baseline: 1.0057x; 1.0057x over previous
"""Embedding lookup (gather of rows) distributed over 8 NeuronCores.

Full problem: x:[1, 8192] int token ids, weights:[50257, 768] f32.
Output: weights[x[0]] -> [8192, 768] f32.

Sharding: data-parallel over the sequence dim. Each of the 8 cores gets
1024 token ids plus a full replica of the embedding table and gathers its
own rows with indirect DMA (DRAM -> SBUF; the HW consumes one index per
partition per indirect DMA, so each gather moves 128 rows), then writes
its contiguous [1024, 768] output slice back to DRAM. No collectives;
the host concatenates the 8 slices.

Raw Bass, no Block(): the engine streams (SP: HWDGE idx load + half the
write-outs; ACT: the other write-outs; gpsimd: the SWDGE gathers, split
over two SWDGE queues) are emitted straight into the main block with
manual semaphores and no end-of-kernel all-engine barrier. Semaphore
reset for re-execution is provided by the runtime's own postamble (it
zeroes all 256 semaphores after every execution).
"""

import numpy as np

import concourse.bass as bass
import concourse.mybir as mybir
from concourse import bacc
from concourse.bass_utils import run_bass_kernel_spmd

VOCAB = 50257
EMBED = 768
SEQ = 8192
N_CORES = 8
TOK_PER_CORE = SEQ // N_CORES  # 1024
J = TOK_PER_CORE // 128  # 8 gathers of 128 rows each


def build_nc():
    nc = bacc.Bacc(
        "TRN2",
        target_bir_lowering=False,
        debug=False,
        num_devices=N_CORES,
        num_swdge_queues=2,
    )
    idx = nc.dram_tensor(
        "idx", [1, TOK_PER_CORE], mybir.dt.int32, kind="ExternalInput"
    )
    w = nc.dram_tensor("w", [VOCAB, EMBED], mybir.dt.float32, kind="ExternalInput")
    out = nc.dram_tensor(
        "out", [TOK_PER_CORE, EMBED], mybir.dt.float32, kind="ExternalOutput"
    )
    # partition p owns tokens p*J .. p*J+J-1 -> out rows p*J+j
    out_pjd = out.ap().rearrange("(p j) d -> p (j d)", p=128)
    idx_re = idx.ap().rearrange("a (p j) -> (a p) j", p=128)

    with (
        nc.sbuf_tensor("idx_sb", [128, J], mybir.dt.int32) as idx_sb,
        nc.sbuf_tensor("gbuf", [128, J * EMBED], mybir.dt.float32) as gbuf,
        nc.semaphore("isem") as isem,
        nc.semaphore("wsem") as wsem,
    ):
        gsems = [nc.alloc_semaphore(f"gsem{g}") for g in range(J)]

        gp = nc.gpsimd
        sp = nc.sync

        # idx load on SP (HWDGE: lower first-byte latency); gathers on gpsimd.
        sp.dma_start(idx_sb[:], idx_re).then_inc(isem, 16)
        gp.wait_ge(isem, 16)
        for j in range(J):
            bi = gp.indirect_dma_start(
                out=gbuf[:, j * EMBED : (j + 1) * EMBED],
                out_offset=None,
                in_=w.ap(),
                in_offset=bass.IndirectOffsetOnAxis(ap=idx_sb[:, j : j + 1], axis=0),
            )
            bi.then_inc(gsems[j], 16)
            if j % 2 == 1:
                # Alternate gathers onto the second SWDGE queue so the two
                # descriptor rings feed the SDMA engines in parallel.
                (bi.instruction if hasattr(bi, "instruction") else bi).queue = (
                    "qPoolDynamic1"
                )

        # Write-outs alternate between the two HWDGE queues (SP and ACT) so
        # descriptor rings and completion receipts don't pile on one queue.
        act = nc.scalar
        for j in range(J):
            eng = sp if j % 2 == 0 else act
            eng.wait_ge(gsems[j], 16)
            eng.dma_start(
                out_pjd[:, j * EMBED : (j + 1) * EMBED],
                gbuf[:, j * EMBED : (j + 1) * EMBED],
            ).then_inc(wsem, 16)
        sp.wait_ge(wsem, J * 16)

    # Drop the const-AP prime memsets Bass emits unconditionally in its
    # prologue — nothing in this kernel reads them, and they extend the
    # measured kernel span at the front.
    entry = nc.m.functions[0].blocks[0]
    dead = [
        i
        for i in entry.instructions
        if isinstance(i, mybir.InstMemset)
        and i.outs
        and str(getattr(i.outs[0], "memref", "")).startswith("const-")
    ]
    for i in dead:
        entry.instructions.remove(i)

    nc.compile()
    return nc


def kernel(x, weights):
    x_np = np.ascontiguousarray(np.asarray(x).reshape(-1).astype(np.int32))
    w_np = np.ascontiguousarray(np.asarray(weights), dtype=np.float32)
    assert x_np.shape == (SEQ,) and w_np.shape == (VOCAB, EMBED)

    nc = build_nc()
    in_maps = [
        {
            "idx": x_np[k * TOK_PER_CORE : (k + 1) * TOK_PER_CORE].reshape(
                1, TOK_PER_CORE
            ),
            "w": w_np,
        }
        for k in range(N_CORES)
    ]
    res = run_bass_kernel_spmd(nc, in_maps, core_ids=list(range(N_CORES)))
    return np.concatenate([r["out"] for r in res.results], axis=0)
